# revision 37
# baseline (speedup 1.0000x reference)
"""BlockDCTSandwich Trainium2 kernel.

The whole op (blockify -> 8x8 DCT -> zigzag gather -> Linear(64,64) -> IDCT
-> deblockify) is a single fused 64x64 linear map per 8x8 block:
    out_vec = M @ x_vec + c,  M = kron(D^T,D^T) @ W @ G @ kron(D,D),
    c = kron(D^T,D^T) @ bias

Data-parallel over batch: one batch element (16 channels, 16.78 MB) per
NeuronCore. Work unit = a PAIR of [128, 512] image tiles (256 rows), software
pipelined so the DVE (the bottleneck engine, 2 stream-transpose passes over
all data) never stalls. Per pair (rows r = 8*hb + n, cols w = 8*wb + m,
wb = s*32 + sw, m = 2*cc + m0):

  load   natural rows (partition = r), SWDGE cast f32 -> bf16
  fwd    16 PE transposes (bf16, tile_position packs m0) -> psT PSUM bf16
            psT[m0*64+wb, cc*256 + t*128 + r] = x_t[r, wb*8 + cc*2 + m0]
  Z      1 DVE stream-transpose, 1024 cols (strided PSUM view) -> Z bf16
            Z[pi, (t*16+hb)*32 + sw] = x_t[hb*8+n, (s*32+sw)*8 + m]
            pi = m0*64 + s*32 + n*4 + cc
  mm     2 bf16 matmuls, stationary LT = M blockdiag over s -> ps PSUM f32
  T_a    1 DVE stream-transpose, 1024 cols              -> OYs SBUF f32
  inv    8 PE transposes (f32)                          -> psO PSUM f32
  evac   ACT copies (free-dim shuffle, cast) -> OXP bf16 -> store bf16
         (host upcasts the gathered output to f32)

PSUM budget: psT(1 bank) + ps(2) + psO(1 per tile) each double-buffered = 8.
Self-contained: hardcodes shapes x=(8,16,512,512) f32, W=(64,64), bias=(64,).
"""

import sys

import numpy as np

if "/opt/trn_rl_repo" not in sys.path:
    sys.path.insert(0, "/opt/trn_rl_repo")

_B = 8
_NCORES = 8


def _dct_matrix(b):
    n = np.arange(b)
    k = n[:, None]
    Dm = np.sqrt(2.0 / b) * np.cos(np.pi * (2 * n[None, :] + 1) * k / (2 * b))
    Dm[0] *= 1.0 / np.sqrt(2.0)
    return Dm


def _build_idx(b):
    def to_key(x):
        s = x[0] + x[1]
        o = b * b * s
        if s % 2 == 1:
            o += x[0]
        else:
            o -= x[0]
        return o

    coords = sorted(([i, j] for i in range(b) for j in range(b)), key=to_key)
    arr = np.array(coords).reshape(b, b, 2)
    return (np.arange(b)[None, :] * arr[..., 0] + arr[..., 1]).reshape(-1)


def _consts(W, bias):
    """Fused 64x64 map M as a 128x128 stationary lhsT (blockdiag over the
    column-half parity s), plus the bias image constant c.

    Partition encoding on both matmul sides: a = m0*64 + s*32 + n*4 + cc,
    with block-local coords (n, m), m = 2*cc + m0.
    """
    D = _dct_matrix(_B)
    idx = _build_idx(_B)
    G = np.zeros((64, 64))
    G[np.arange(64), idx] = 1.0
    M = np.kron(D.T, D.T) @ W.astype(np.float64) @ G @ np.kron(D, D)
    c = np.kron(D.T, D.T) @ bias.astype(np.float64)

    enc = np.arange(128)
    loc = 8 * ((enc >> 2) & 7) + 2 * (enc & 3) + (enc >> 6)
    spar = (enc >> 5) & 1
    LT = M[np.ix_(loc, loc)].T * (spar[:, None] == spar[None, :])
    return LT.astype(np.float32), c


_NC_CACHE = {}


def _build_nc():
    if "nc" in _NC_CACHE:
        return _NC_CACHE["nc"]
    import concourse.bass as bass
    import concourse.mybir as mybir
    from concourse import bacc
    from concourse.tile import TileContext

    f32 = mybir.dt.float32
    bf16 = mybir.dt.bfloat16
    ds = bass.ds

    nc = bacc.Bacc("TRN2", target_bir_lowering=False, debug=False,
                   num_devices=_NCORES)
    xin = nc.dram_tensor("xin", [8192, 512], f32, kind="ExternalInput")
    ltw = nc.dram_tensor("ltw", [128, 128], bf16, kind="ExternalInput")
    idw = nc.dram_tensor("idw", [128, 128], bf16, kind="ExternalInput")
    idwf = nc.dram_tensor("idwf", [128, 128], f32, kind="ExternalInput")
    yout = nc.dram_tensor("yout", [8192, 512], bf16, kind="ExternalOutput")

    xin_ap = xin.ap()
    yout_ap = yout.ap()

    with TileContext(nc) as tc:
        with (
            tc.tile_pool(name="wp", bufs=1) as wp,
            tc.tile_pool(name="io", bufs=4) as iop,
            tc.tile_pool(name="wk", bufs=6) as wk,
            tc.tile_pool(name="psp", bufs=2, space="PSUM") as psp,
        ):
            lt_sb = wp.tile([128, 128], bf16)
            nc.sync.dma_start(out=lt_sb[:, :], in_=ltw.ap())
            id_sb = wp.tile([128, 128], bf16, tag="id_sb")
            nc.sync.dma_start(out=id_sb[:, :], in_=idw.ap())
            idf_sb = wp.tile([128, 128], f32, tag="idf_sb")
            nc.sync.dma_start(out=idf_sb[:, :], in_=idwf.ap())

            # Software-pipelined flat loop over 32 tile-PAIRS (each pair =
            # two [128, 512] tiles, 1024-wide DVE transposes to amortize the
            # PSUM-access init). Steady-state issue per iter p:
            #   mm_p | Z_{p+1} | inv_{p-1} | T_a_p | fwdT_{p+2} | evac/store
            # DVE (bottleneck) order [Z_{p+1}, T_a_p] never stalls: mm_p
            # (427ns) completes inside Z_{p+1} (1192ns).
            NP = 32
            XBs, psTs, Zs, pss, OYss, psOs = {}, {}, {}, {}, {}, {}

            def load(p):
                if p >= NP or p in XBs:
                    return
                XB = iop.tile([128, 1024], bf16, tag="XB")
                nc.gpsimd.dma_start(
                    out=XB[:, :].rearrange("p (t2 w) -> p t2 w", t2=2),
                    in_=xin_ap[ds(p * 256, 256), :]
                    .rearrange("(t2 r) w -> t2 r w", t2=2, r=128)
                    .transpose([1, 0, 2]),
                )
                XBs[p] = XB

            def fwd_psT(p):
                if p >= NP:
                    return
                load(p)
                psT = psp.tile([128, 1024], bf16, tag="psT")
                for t in range(2):
                    Xs = XBs.pop(p)[:, ds(t * 512, 512)] if t == 1 \
                        else XBs[p][:, ds(t * 512, 512)]
                    xv = Xs.rearrange("p (wb m) -> p wb m", wb=64, m=8)
                    for m in range(8):
                        cc, m0 = m >> 1, m & 1
                        nc.tensor.transpose(
                            psT[ds(64 * m0, 64),
                                ds(cc * 256 + t * 128, 128)],
                            in_=xv[:, :, m],
                            identity=id_sb[:, :],
                            tile_position=(0, 64 * m0),
                        )
                psTs[p] = psT

            def z_step(p):
                if p >= NP:
                    return
                Z = wk.tile([128, 1024], bf16, tag="Z")
                nc.vector.transpose(
                    out=Z[:, :],
                    in_=psTs.pop(p)[:, :].rearrange(
                        "p (cc thb n) -> p thb n cc", cc=4, thb=32, n=8),
                )
                Zs[p] = Z

            def mm_step(p):
                ps = psp.tile([128, 1024], f32, tag="ps")
                Z = Zs.pop(p)
                for t in range(2):
                    nc.tensor.matmul(ps[:, ds(t * 512, 512)], lt_sb[:, :],
                                     Z[:, ds(t * 512, 512)],
                                     start=True, stop=True)
                pss[p] = ps

            def ta_step(p):
                OYs = wk.tile([128, 1024], f32, tag="OYs")
                nc.vector.transpose(
                    out=OYs[:, :].rearrange("p (cc thb n) -> p thb n cc",
                                            cc=4, thb=32, n=8),
                    in_=pss.pop(p)[:, :],
                )
                OYss[p] = OYs

            def inv_step(p):
                if p < 0:
                    return
                OYs = OYss.pop(p)
                tiles = []
                for t in range(2):
                    psO = psp.tile([128, 512], f32, tag="psO")
                    for cc in range(4):
                        nc.tensor.transpose(
                            psO[:, ds(cc * 128, 128)],
                            in_=OYs[:, ds(cc * 256 + t * 128, 128)],
                            identity=idf_sb[:, :],
                        )
                    tiles.append(psO)
                psOs[p] = tiles

            def evac_store(p):
                if p < 0:
                    return
                OXP = iop.tile([128, 1024], bf16, tag="OXP")
                for t, psO in enumerate(psOs.pop(p)):
                    nc.scalar.copy(
                        OXP[:, ds(t * 512, 512)].rearrange(
                            "p (sw cc m0) -> p cc m0 sw", sw=64, cc=4, m0=2),
                        psO[:, :].rearrange(
                            "p (cc m0 sw) -> p cc m0 sw", cc=4, m0=2, sw=64),
                    )
                    nc.sync.dma_start(
                        out=yout_ap[ds(p * 256 + t * 128, 128), :],
                        in_=OXP[:, ds(t * 512, 512)],
                    )

            # prologue
            load(0)
            load(1)
            fwd_psT(0)
            z_step(0)
            fwd_psT(1)

            for p in range(NP):
                load(p + 2)
                mm_step(p)
                z_step(p + 1)
                inv_step(p - 1)
                ta_step(p)
                fwd_psT(p + 2)
                evac_store(p - 1)
            inv_step(NP - 1)
            evac_store(NP - 1)

    nc.finalize()
    _NC_CACHE["nc"] = nc
    return nc


def run(x, W, bias, trace=False):
    from concourse.bass_utils import run_bass_kernel_spmd
    import ml_dtypes

    x = np.ascontiguousarray(np.asarray(x, dtype=np.float32))
    W = np.asarray(W, dtype=np.float32)
    bias = np.asarray(bias, dtype=np.float32)
    assert x.shape == (8, 16, 512, 512), x.shape

    LT, c = _consts(W, bias)
    nc = _build_nc()
    identf = np.eye(128, dtype=np.float32)
    ident = identf.astype(ml_dtypes.bfloat16)
    LTh = LT.astype(ml_dtypes.bfloat16)
    in_maps = [
        {"xin": np.ascontiguousarray(x[i].reshape(8192, 512)), "ltw": LTh,
         "idw": ident, "idwf": identf}
        for i in range(_NCORES)
    ]
    res = run_bass_kernel_spmd(nc, in_maps, core_ids=list(range(_NCORES)),
                               trace=trace)
    out = np.stack(
        [np.asarray(res.results[i]["yout"], dtype=np.float32)
         .reshape(16, 512, 512) for i in range(_NCORES)]
    )
    if np.any(c):
        cimg = np.tile(c.reshape(8, 8), (64, 64)).astype(np.float32)
        out = out + cimg[None, None]
    return out.astype(np.float32), res


def kernel(x, W, bias):
    out, _ = run(x, W, bias, trace=False)
    return out


# revision 38
# speedup vs baseline: 1.1027x; 1.1027x over previous
"""BlockDCTSandwich Trainium2 kernel.

The whole op (blockify -> 8x8 DCT -> zigzag gather -> Linear(64,64) -> IDCT
-> deblockify) is a single fused 64x64 linear map per 8x8 block:
    out_vec = M @ x_vec + c,  M = kron(D^T,D^T) @ W @ G @ kron(D,D),
    c = kron(D^T,D^T) @ bias

Data-parallel over batch: one batch element (16 channels, 16.78 MB) per
NeuronCore. Work unit = a PAIR of [128, 512] image tiles (256 rows), software
pipelined so the DVE (the bottleneck engine, 2 stream-transpose passes over
all data) never stalls. Per pair (rows r = 8*hb + n, cols w = 8*wb + m,
wb = s*32 + sw, m = 2*cc + m0):

  load   natural rows (partition = r), SWDGE cast f32 -> bf16
  fwd    16 PE transposes (bf16, tile_position packs m0) -> psT PSUM bf16
            psT[m0*64+wb, cc*256 + t*128 + r] = x_t[r, wb*8 + cc*2 + m0]
  Z      1 DVE stream-transpose, 1024 cols (strided PSUM view) -> Z bf16
            Z[pi, (t*16+hb)*32 + sw] = x_t[hb*8+n, (s*32+sw)*8 + m]
            pi = m0*64 + s*32 + n*4 + cc
  mm     2 bf16 matmuls, stationary LT = M blockdiag over s -> ps PSUM f32
  T_a    1 DVE stream-transpose, 1024 cols              -> OYs SBUF f32
  inv    8 PE transposes (f32)                          -> psO PSUM f32
  evac   ACT copies (free-dim shuffle, cast) -> OXP bf16 -> store bf16
         (host upcasts the gathered output to f32)

PSUM budget: psT(1 bank) + ps(2) + psO(1 per tile) each double-buffered = 8.
Self-contained: hardcodes shapes x=(8,16,512,512) f32, W=(64,64), bias=(64,).
"""

import sys

import numpy as np

if "/opt/trn_rl_repo" not in sys.path:
    sys.path.insert(0, "/opt/trn_rl_repo")

_B = 8
_NCORES = 8


def _dct_matrix(b):
    n = np.arange(b)
    k = n[:, None]
    Dm = np.sqrt(2.0 / b) * np.cos(np.pi * (2 * n[None, :] + 1) * k / (2 * b))
    Dm[0] *= 1.0 / np.sqrt(2.0)
    return Dm


def _build_idx(b):
    def to_key(x):
        s = x[0] + x[1]
        o = b * b * s
        if s % 2 == 1:
            o += x[0]
        else:
            o -= x[0]
        return o

    coords = sorted(([i, j] for i in range(b) for j in range(b)), key=to_key)
    arr = np.array(coords).reshape(b, b, 2)
    return (np.arange(b)[None, :] * arr[..., 0] + arr[..., 1]).reshape(-1)


def _consts(W, bias):
    """Fused 64x64 map M as a 128x128 stationary lhsT (blockdiag over the
    column-half parity s), plus the bias image constant c.

    Partition encoding on both matmul sides: a = m0*64 + s*32 + n*4 + cc,
    with block-local coords (n, m), m = 2*cc + m0.
    """
    D = _dct_matrix(_B)
    idx = _build_idx(_B)
    G = np.zeros((64, 64))
    G[np.arange(64), idx] = 1.0
    M = np.kron(D.T, D.T) @ W.astype(np.float64) @ G @ np.kron(D, D)
    c = np.kron(D.T, D.T) @ bias.astype(np.float64)

    enc = np.arange(128)
    loc = 8 * (enc & 7) + 2 * ((enc >> 3) & 3) + (enc >> 6)
    spar = (enc >> 5) & 1
    LT = M[np.ix_(loc, loc)].T * (spar[:, None] == spar[None, :])
    return LT.astype(np.float32), c


_NC_CACHE = {}


def _build_nc():
    if "nc" in _NC_CACHE:
        return _NC_CACHE["nc"]
    import concourse.bass as bass
    import concourse.mybir as mybir
    from concourse import bacc
    from concourse.tile import TileContext

    f32 = mybir.dt.float32
    bf16 = mybir.dt.bfloat16
    ds = bass.ds

    nc = bacc.Bacc("TRN2", target_bir_lowering=False, debug=False,
                   num_devices=_NCORES)
    xin = nc.dram_tensor("xin", [8192, 512], f32, kind="ExternalInput")
    ltw = nc.dram_tensor("ltw", [128, 128], bf16, kind="ExternalInput")
    idw = nc.dram_tensor("idw", [128, 128], bf16, kind="ExternalInput")
    idwf = nc.dram_tensor("idwf", [128, 128], f32, kind="ExternalInput")
    yout = nc.dram_tensor("yout", [8192, 512], bf16, kind="ExternalOutput")

    xin_ap = xin.ap()
    yout_ap = yout.ap()

    with TileContext(nc) as tc:
        with (
            tc.tile_pool(name="wp", bufs=1) as wp,
            tc.tile_pool(name="io", bufs=4) as iop,
            tc.tile_pool(name="wk", bufs=6) as wk,
            tc.tile_pool(name="psp", bufs=2, space="PSUM") as psp,
        ):
            lt_sb = wp.tile([128, 128], bf16)
            nc.sync.dma_start(out=lt_sb[:, :], in_=ltw.ap())
            id_sb = wp.tile([128, 128], bf16, tag="id_sb")
            nc.sync.dma_start(out=id_sb[:, :], in_=idw.ap())
            idf_sb = wp.tile([128, 128], f32, tag="idf_sb")
            nc.sync.dma_start(out=idf_sb[:, :], in_=idwf.ap())

            # Software-pipelined flat loop over 32 tile-PAIRS (each pair =
            # two [128, 512] tiles, 1024-wide DVE transposes to amortize the
            # PSUM-access init). Steady-state issue per iter p:
            #   mm_p | Z_{p+1} | inv_{p-1} | T_a_p | fwdT_{p+2} | evac/store
            # DVE (bottleneck) order [Z_{p+1}, T_a_p] never stalls: mm_p
            # (427ns) completes inside Z_{p+1} (1192ns).
            NP = 32
            XBs, psTs, Zs, pss, OYss, psOs = {}, {}, {}, {}, {}, {}

            def load(p):
                if p >= NP or p in XBs:
                    return
                XB = iop.tile([128, 1024], bf16, tag="XB")
                for t in range(2):
                    nc.gpsimd.dma_start(
                        out=XB[:, ds(t * 512, 512)],
                        in_=xin_ap[ds(p * 256 + t * 128, 128), :]
                        .rearrange("(hb n) w -> n hb w", hb=16, n=8),
                    )
                XBs[p] = XB

            def fwd_psT(p):
                if p >= NP:
                    return
                load(p)
                psT = psp.tile([128, 1024], bf16, tag="psT")
                for t in range(2):
                    Xs = XBs.pop(p)[:, ds(t * 512, 512)] if t == 1 \
                        else XBs[p][:, ds(t * 512, 512)]
                    xv = Xs.rearrange("p (wb m) -> p wb m", wb=64, m=8)
                    for m in range(8):
                        cc, m0 = m >> 1, m & 1
                        nc.tensor.transpose(
                            psT[ds(64 * m0, 64),
                                ds(t * 512 + cc * 128, 128)],
                            in_=xv[:, :, m],
                            identity=id_sb[:, :],
                            tile_position=(0, 64 * m0),
                        )
                psTs[p] = psT

            def z_step(p):
                if p >= NP:
                    return
                Z = wk.tile([128, 1024], bf16, tag="Z")
                nc.vector.transpose(
                    out=Z[:, :].bitcast(f32),
                    in_=psTs.pop(p)[:, :].bitcast(f32).rearrange(
                        "p (t cc n hbh) -> p t hbh (cc n)",
                        t=2, cc=4, n=8, hbh=8),
                )
                Zs[p] = Z

            def mm_step(p):
                ps = psp.tile([128, 1024], f32, tag="ps")
                Z = Zs.pop(p)
                for t in range(2):
                    nc.tensor.matmul(ps[:, ds(t * 512, 512)], lt_sb[:, :],
                                     Z[:, ds(t * 512, 512)],
                                     start=True, stop=True)
                pss[p] = ps

            def ta_step(p):
                ps = pss.pop(p)
                halves = []
                for t in range(2):
                    OYh = wk.tile([128, 512], f32, tag="OYh")
                    nc.vector.transpose(
                        out=OYh[:, :].rearrange("p (cc hb n) -> p hb cc n",
                                                cc=4, hb=16, n=8),
                        in_=ps[:, ds(t * 512, 512)].rearrange(
                            "p (hbh wbl hb0) -> p hbh hb0 wbl",
                            hbh=8, wbl=32, hb0=2),
                    )
                    halves.append(OYh)
                OYss[p] = halves

            def inv_step(p):
                if p < 0:
                    return
                OYs = OYss.pop(p)
                tiles = []
                for t in range(2):
                    psO = psp.tile([128, 512], f32, tag="psO")
                    for cc in range(4):
                        nc.tensor.transpose(
                            psO[:, ds(cc * 128, 128)],
                            in_=OYs[t][:, ds(cc * 128, 128)],
                            identity=idf_sb[:, :],
                        )
                    tiles.append(psO)
                psOs[p] = tiles

            def evac_store(p):
                if p < 0:
                    return
                OXP = iop.tile([128, 1024], bf16, tag="OXP")
                for t, psO in enumerate(psOs.pop(p)):
                    nc.scalar.copy(
                        OXP[:, ds(t * 512, 512)].rearrange(
                            "p (sw cc m0) -> p cc m0 sw", sw=64, cc=4, m0=2),
                        psO[:, :].rearrange(
                            "p (cc m0 sw) -> p cc m0 sw", cc=4, m0=2, sw=64),
                    )
                    nc.sync.dma_start(
                        out=yout_ap[ds(p * 256 + t * 128, 128), :],
                        in_=OXP[:, ds(t * 512, 512)],
                    )

            # prologue
            load(0)
            load(1)
            fwd_psT(0)
            z_step(0)
            fwd_psT(1)

            for p in range(NP):
                load(p + 2)
                mm_step(p)
                z_step(p + 1)
                inv_step(p - 1)
                ta_step(p)
                fwd_psT(p + 2)
                evac_store(p - 1)
            inv_step(NP - 1)
            evac_store(NP - 1)

    nc.finalize()
    _NC_CACHE["nc"] = nc
    return nc


def run(x, W, bias, trace=False):
    from concourse.bass_utils import run_bass_kernel_spmd
    import ml_dtypes

    x = np.ascontiguousarray(np.asarray(x, dtype=np.float32))
    W = np.asarray(W, dtype=np.float32)
    bias = np.asarray(bias, dtype=np.float32)
    assert x.shape == (8, 16, 512, 512), x.shape

    LT, c = _consts(W, bias)
    nc = _build_nc()
    identf = np.eye(128, dtype=np.float32)
    ident = identf.astype(ml_dtypes.bfloat16)
    LTh = LT.astype(ml_dtypes.bfloat16)
    in_maps = [
        {"xin": np.ascontiguousarray(x[i].reshape(8192, 512)), "ltw": LTh,
         "idw": ident, "idwf": identf}
        for i in range(_NCORES)
    ]
    res = run_bass_kernel_spmd(nc, in_maps, core_ids=list(range(_NCORES)),
                               trace=trace)
    out = np.stack(
        [np.asarray(res.results[i]["yout"], dtype=np.float32)
         .reshape(16, 512, 512) for i in range(_NCORES)]
    )
    if np.any(c):
        cimg = np.tile(c.reshape(8, 8), (64, 64)).astype(np.float32)
        out = out + cimg[None, None]
    return out.astype(np.float32), res


def kernel(x, W, bias):
    out, _ = run(x, W, bias, trace=False)
    return out
